# revision 4
# baseline (speedup 1.0000x reference)
"""Grouped MLP (MoE expert-parallel) Trainium2 kernel — v3.

Problem: hidden_states [131072, 1024] f32, 8 experts each owning a contiguous
16384-token block; per expert: SwiGLU MLP with gate_up [1024, 1024] and
down [512, 1024].

Sharding: expert-parallel — core e computes expert e's token block entirely
locally (no collectives). Inputs are sliced host-side, outputs concatenated.

v3 design: the PE does ONLY matmuls (bf16, 1 cyc/row); both transposes run on
the DMA XBAR (InstDmaTransposeAnt, 2-byte dtype); x is cast f32->bf16 during
the HBM load by the SWDGE (gpsimd) cast DMA. DMA work is batched in G-tile
groups (G*128 tokens) to amortize per-DMA issue latency and semaphore
propagation (~900ns each), and issue paths are spread across engines
(loads: Pool/SWDGE, x-transpose + store: SP, h-transpose: ACT).

Per-core, per 128-token tile (within a G-tile DMA group):
  - SWDGE load+cast: x [128, G, 1024] f32 HBM -> bf16 SBUF
  - SP DMA-transpose: -> xT [128(h), G*8, 128(t)] bf16
  - mm1: 2 PSUM [128, 512] f32, 8 accumulating bf16 matmuls each
  - ACT Sigmoid(gate_ps) -> s f32; DVE s*up -> t1; DVE t1*gate -> h bf16
  - ACT DMA-transpose: h [128, G, 512] -> hT [128(i), G*4, 128(t)] bf16
  - mm2: 2 PSUM [128, 512] f32, 4 accumulating bf16 matmuls each
  - ACT/DVE copy PSUM -> o [128, G, 1024] f32, SP DMA store

bf16 operands give ~4e-3 relative error (budget 2e-2).
"""

import numpy as np

E = 8
H = 1024
I = 512
T_PER_CORE = 16384
N_CORES = 8
G = 2  # tiles per DMA group

_cache = {}


def _build_nc(n_tiles, g=G):
    import os

    import concourse.mybir as mybir
    import concourse.tile as tile
    from concourse import bacc
    from concourse.masks import make_identity

    variant = os.environ.get("KV3_VARIANT", "")
    assert n_tiles % g == 0
    n_groups = n_tiles // g

    f32 = mybir.dt.float32
    bf16 = mybir.dt.bfloat16

    nc = bacc.Bacc(None, target_bir_lowering=False)
    n_tok = n_tiles * 128
    x = nc.dram_tensor("x", [n_tok, H], f32, kind="ExternalInput")
    w1 = nc.dram_tensor("w1", [H, 2 * I], f32, kind="ExternalInput")
    w2 = nc.dram_tensor("w2", [I, H], f32, kind="ExternalInput")
    out = nc.dram_tensor("out", [n_tok, H], f32, kind="ExternalOutput")

    with tile.TileContext(nc) as tc:
        with (
            tc.tile_pool(name="const", bufs=1) as const,
            tc.tile_pool(name="xinf", bufs=3) as xinf,
            tc.tile_pool(name="xin", bufs=3) as xin,
            tc.tile_pool(name="xtp", bufs=3) as xtp,
            tc.tile_pool(name="sp", bufs=4) as sp,
            tc.tile_pool(name="hp", bufs=3) as hp,
            tc.tile_pool(name="htp", bufs=3) as htp,
            tc.tile_pool(name="outp", bufs=3) as outp,
            tc.tile_pool(name="mm1_ps", bufs=3 if os.environ.get("KV3_VARIANT") == "b" else 4, space="PSUM") as mm1_ps_pool,
            tc.tile_pool(name="mm2_ps", bufs=3 if os.environ.get("KV3_VARIANT") == "b" else 4, space="PSUM") as mm2_ps_pool,
            tc.tile_pool(name="tp_ps", bufs=2, space="PSUM") as tp_ps_pool,
        ):
            # Resident weights: load f32 (same-dtype SWDGE), cast to bf16 on
            # engines (casting SWDGE DMAs are unreliable on HW — see
            # stage_load). Staging tiles stay allocated; SBUF has room.
            w1_sb = const.tile([128, H // 128, 2 * I], bf16)
            w2_sb = const.tile([128, I // 128, H], bf16)

            def load_weights():
                w1_f = const.tile([128, H // 128, 2 * I], f32)
                nc.gpsimd.dma_start(
                    w1_f[:], w1.ap().rearrange("(ho p) f -> p ho f", p=128)
                )
                w2_f = const.tile([128, I // 128, H], f32)
                nc.gpsimd.dma_start(
                    w2_f[:], w2.ap().rearrange("(io p) f -> p io f", p=128)
                )
                nc.vector.tensor_copy(w1_sb[:, :4, :], w1_f[:, :4, :])
                nc.scalar.copy(w1_sb[:, 4:, :], w1_f[:, 4:, :])
                nc.gpsimd.tensor_copy(w2_sb[:], w2_f[:])

            if variant == "b":
                ident = const.tile([128, 128], bf16)
                make_identity(nc, ident)

            xbf_d, xT_d, mm1_d, h_d, hT_d, o_d = {}, {}, {}, {}, {}, {}

            xf_d = {}

            load_eng = {"hwl": nc.scalar, "swdge": nc.gpsimd}.get(
                os.environ.get("KV3_LOAD", ""), nc.sync
            )

            def stage_load(gi):
                # SWDGE f32->bf16 cast loads corrupt scattered partition rows
                # on HW (completion-sem miscount for casting DMAs), so load
                # f32 (same-dtype SWDGE is solid) and cast on the idle gpsimd.
                x_f = xinf.tile([128, g, H], f32, tag="xf", name="x_f")
                load_eng.dma_start(
                    x_f[:],
                    x.ap()[gi * g * 128 : (gi + 1) * g * 128, :].rearrange(
                        "(c p) h -> p c h", p=128
                    ),
                )
                xf_d[gi] = x_f

            def stage_cast(gi):
                # One iteration after the load, so the in-order Pool engine
                # never waits here (which would stall the next load's issue).
                x_f = xf_d.pop(gi)
                x_t = xin.tile([128, g, H], bf16, tag="x", name="x_t")
                cast_eng = {"pool": nc.gpsimd, "act": nc.scalar}.get(
                    os.environ.get("KV3_CAST", ""), nc.vector
                )
                cast_eng.tensor_copy(x_t[:], x_f[:])
                xbf_d[gi] = x_t

            def stage_xtrans(gi):
                x_t = xbf_d.pop(gi)
                xT = xtp.tile([128, g * (H // 128), 128], bf16, tag="xT")
                if variant == "b":
                    for c in range(g):
                        tp_ps = tp_ps_pool.tile([128, 4, 128], bf16, tag="tp", name="tp")
                        for k in range(4):
                            nc.tensor.transpose(
                                tp_ps[:, k, :], x_t[:, c, k * 128 : (k + 1) * 128], ident
                            )
                        nc.scalar.copy(xT[:, c * 8 : c * 8 + 4, :], tp_ps[:])
                        tp_ps2 = tp_ps_pool.tile([128, 4, 128], bf16, tag="tp", name="tp2")
                        for k in range(4):
                            nc.tensor.transpose(
                                tp_ps2[:, k, :], x_t[:, c, 512 + k * 128 : 512 + (k + 1) * 128], ident
                            )
                        nc.scalar.copy(xT[:, c * 8 + 4 : c * 8 + 8, :], tp_ps2[:])
                else:
                    xtp_eng = nc.sync if os.environ.get("KV3_XTP", "") == "sp" else nc.scalar
                    xtp_eng.dma_start(
                        xT[:], x_t[:].rearrange("p c h -> p (c h)"), transpose=True
                    )
                xT_d[gi] = xT

            def stage_mm1(t):
                gi, c = divmod(t, g)
                xT = xT_d[gi]
                ps_pair = []
                for f in range(2):
                    ps = mm1_ps_pool.tile([128, 512], f32, tag="mm1")
                    for k in range(H // 128):
                        nc.tensor.matmul(
                            ps[:],
                            xT[:, c * (H // 128) + k, :],
                            w1_sb[:, k, f * 512 : (f + 1) * 512],
                            start=(k == 0),
                            stop=(k == H // 128 - 1),
                        )
                    ps_pair.append(ps)
                mm1_d[t] = ps_pair
                if c == g - 1:
                    xT_d.pop(gi)

            def stage_swiglu(t):
                gi, c = divmod(t, g)
                gate_ps, up_ps = mm1_d.pop(t)
                s = sp.tile([128, 512], f32, tag="s")
                nc.scalar.activation(
                    s[:], gate_ps[:], mybir.ActivationFunctionType.Sigmoid
                )
                t1 = sp.tile([128, 512], f32, tag="t1")
                nc.vector.tensor_mul(t1[:], s[:], up_ps[:])
                if c == 0:
                    h_d[gi] = hp.tile([128, g, 512], bf16, tag="h", name="h")
                nc.vector.tensor_mul(h_d[gi][:, c, :], t1[:], gate_ps[:])

            def stage_htrans(gi):
                h = h_d.pop(gi)
                hT = htp.tile([128, g * (I // 128), 128], bf16, tag="hT")
                nc.scalar.dma_start(hT[:], h[:].rearrange("p c i -> p (c i)"), transpose=True)
                hT_d[gi] = hT

            def stage_mm2(t):
                gi, c = divmod(t, g)
                hT = hT_d[gi]
                if c == 0:
                    o_d[gi] = outp.tile([128, g, H], f32, tag="o", name="o")
                o_t = o_d[gi]
                for f in range(2):
                    ps2 = mm2_ps_pool.tile([128, 512], f32, tag="mm2")
                    for k in range(I // 128):
                        nc.tensor.matmul(
                            ps2[:],
                            hT[:, c * (I // 128) + k, :],
                            w2_sb[:, k, f * 512 : (f + 1) * 512],
                            start=(k == 0),
                            stop=(k == I // 128 - 1),
                        )
                    if f == 0:
                        nc.scalar.copy(o_t[:, c, :512], ps2[:])
                    else:
                        nc.vector.tensor_copy(o_t[:, c, 512:], ps2[:])
                if c == g - 1:
                    hT_d.pop(gi)

            def stage_store(gi):
                o_t = o_d.pop(gi)
                nc.sync.dma_start(
                    out.ap()[gi * g * 128 : (gi + 1) * g * 128, :].rearrange(
                        "(c p) h -> p c h", p=128
                    ),
                    o_t[:],
                )

            # Software pipeline over G-tile groups. The tile scheduler
            # re-orders per-engine; the dependency slack is what matters:
            # each cross-engine product is ready a full group-iteration
            # (~10us) before its consumer.
            # Every DMA consumer lags its producer by one full iteration so
            # no sequencer ever holds a long sem-wait that head-of-line
            # blocks later ready work (SP: xtrans+store, Pool: load+cast).
            for j in range(n_groups + 6):
                if j < n_groups:
                    stage_load(j)
                if j == 0:
                    # After the first x loads, so those transfers lead the
                    # 18us of weight traffic on the DMA engines.
                    load_weights()
                if 1 <= j <= n_groups:
                    stage_cast(j - 1)
                if 2 <= j <= n_groups + 1:
                    stage_xtrans(j - 2)
                if 4 <= j <= n_groups + 3:
                    stage_htrans(j - 4)
                if 6 <= j <= n_groups + 5:
                    stage_store(j - 6)
                if 5 <= j <= n_groups + 4:
                    for c in range(g):
                        stage_mm2((j - 5) * g + c)
                if 3 <= j <= n_groups + 2:
                    for c in range(g):
                        stage_mm1((j - 3) * g + c)
                        stage_swiglu((j - 3) * g + c)

    nc.compile()
    return nc


def _get_nc(n_tiles):
    if n_tiles not in _cache:
        _cache[n_tiles] = _build_nc(n_tiles)
    return _cache[n_tiles]


def kernel(hidden_states, gate_up_proj, down_proj, num_tokens_per_expert):
    sizes = np.asarray(num_tokens_per_expert)
    offsets = np.concatenate([[0], np.cumsum(sizes)])
    uniform = (
        sizes.shape[0] == E
        and np.all(sizes == T_PER_CORE)
        and hidden_states.shape == (E * T_PER_CORE, H)
    )
    if not uniform:
        # Fallback: host-side numpy (routing metadata other than the
        # compiled uniform case).
        outs = []
        for e in range(sizes.shape[0]):
            xe = hidden_states[offsets[e] : offsets[e + 1]].astype(np.float32)
            merged = xe @ gate_up_proj[e]
            gate, up = merged[:, :I], merged[:, I:]
            he = (gate / (1.0 + np.exp(-gate))) * up
            outs.append(he @ down_proj[e])
        return np.concatenate(outs, axis=0).astype(hidden_states.dtype)

    from concourse.bass_utils import run_bass_kernel_spmd

    nc = _get_nc(T_PER_CORE // 128)
    hs = np.ascontiguousarray(np.asarray(hidden_states, dtype=np.float32))
    w1 = np.ascontiguousarray(np.asarray(gate_up_proj, dtype=np.float32))
    w2 = np.ascontiguousarray(np.asarray(down_proj, dtype=np.float32))
    in_maps = [
        {
            "x": hs[e * T_PER_CORE : (e + 1) * T_PER_CORE],
            "w1": w1[e],
            "w2": w2[e],
        }
        for e in range(N_CORES)
    ]
    res = run_bass_kernel_spmd(nc, in_maps, core_ids=list(range(N_CORES)))
    return np.concatenate([r["out"] for r in res.results], axis=0)
